# revision 12
# baseline (speedup 1.0000x reference)
"""3-branch 2-layer GAT classifier on 8 Trainium2 NeuronCores (Bass/Tile).

Strategy (matches the edge-cut sharding hint):
- Nodes (and their incoming edges) are sharded contiguously across the 8
  cores; each core owns N/8 destination nodes for both GAT layers.
- Layer 1: every core computes the full node "table" [feat | el | er] =
  x @ [W | W.al | W.ar] with the tensor engine (replicating this cheap
  matmul avoids any halo exchange for layer 1), writes it to local DRAM,
  then processes its edges: per-destination-node degree-bucketed tiles,
  row-gather of source features via indirect DMA, edge softmax on the
  ACT/DVE engines, fused multiply-accumulate aggregation on DVE.
- Layer 2: each core computes its shard of the layer-2 table from its
  aggregated h1 rows, the shards are AllGathered (the halo exchange),
  and the layer-2 edge phase runs like layer 1.
- Readout: per-graph mean via an indicator-matrix matmul accumulated in
  PSUM, partial sums AllReduced across cores, then the small MLP head is
  computed (replicated) on every core.

Host-side work is integer-only index preprocessing (degree bucketing,
CSR, permutations) plus layout (transpose/pad) of the input arrays; all
floating-point math runs on the NeuronCores.
"""

import os
import numpy as np
from contextlib import ExitStack

import concourse.bass as bass
import concourse.tile as tile
from concourse import bacc, mybir
from concourse import bass_utils
from concourse.masks import make_identity

AF = mybir.ActivationFunctionType
ALU = mybir.AluOpType
F32 = mybir.dt.float32
I32 = mybir.dt.int32

NC = 8          # cores
P = 128         # partitions
NEG_BIG = -1.0e30

LAST_EXEC_NS = None

_CACHE = {}


# ----------------------------------------------------------------------------
# Host-side integer preprocessing
# ----------------------------------------------------------------------------

def _preprocess(src, dst, gid, N):
    Ncore = N // NC
    TILES = (Ncore + P - 1) // P
    NT = TILES * P

    deg = np.bincount(dst, minlength=N)

    # CSR of edges grouped by destination
    eorder = np.argsort(dst, kind="stable")
    srcs_sorted = src[eorder].astype(np.int64)
    rowptr = np.zeros(N + 1, np.int64)
    rowptr[1:] = np.cumsum(deg)

    # per-core degree-sorted node order
    node_order = np.full((NC, NT), -1, np.int64)
    for c in range(NC):
        d = deg[c * Ncore:(c + 1) * Ncore]
        o = np.argsort(-d, kind="stable")
        node_order[c, :Ncore] = c * Ncore + o

    # common per-tile padded degree K_t (max across cores)
    degp = np.zeros((NC, NT), np.int64)
    for c in range(NC):
        real = node_order[c] >= 0
        degp[c, real] = deg[node_order[c][real]]
    K_t = degp.reshape(NC, TILES, P).max(axis=(0, 2))
    K_t = np.maximum(K_t, 1).astype(np.int64)
    S1 = int(K_t.sum())
    off_t = np.zeros(TILES + 1, np.int64)
    off_t[1:] = np.cumsum(K_t)

    # position of each node in the (post-allgather) layer-2 table;
    # each core's shard carries NT real rows plus one dummy row at its end,
    # so global position = c * (NT + 1) + rank. The dummy index points at
    # core 0's dummy row (position NT).
    NTS = NT + 1
    pos2 = np.zeros(N, np.int64)
    for c in range(NC):
        real = node_order[c] >= 0
        pos2[node_order[c][real]] = c * NTS + np.nonzero(real)[0]

    T1TILES = (N + P) // P        # ceil((N+1)/P): row N is the dummy
    T1ROWS = T1TILES * P
    dummy1 = N
    dummy2 = NT

    idx1 = np.full((NC, P, S1), dummy1, np.int32)
    idx2 = np.full((NC, P, S1), dummy2, np.int32)
    own1 = np.full((NC, P, TILES), dummy1, np.int32)
    own2 = np.full((NC, P, TILES), dummy2, np.int32)
    Mmat = np.zeros((NC, TILES, P, P), np.float32)
    scat = np.zeros((NC, P, 1), np.int32)

    g_lo = np.zeros(NC, np.int64)
    for c in range(NC):
        g_lo[c] = gid[c * Ncore]
        g_hi = gid[(c + 1) * Ncore - 1]
        assert g_hi - g_lo[c] + 1 <= P, "graph window exceeds 128"
        scat[c, :, 0] = g_lo[c] + np.arange(P)
        for t in range(TILES):
            K = K_t[t]
            for p in range(P):
                n = node_order[c, t * P + p]
                if n < 0:
                    continue
                dn = deg[n]
                es = srcs_sorted[rowptr[n]:rowptr[n] + dn]
                idx1[c, p, off_t[t]:off_t[t] + dn] = es
                idx2[c, p, off_t[t]:off_t[t] + dn] = pos2[es]
                own1[c, p, t] = n
                own2[c, p, t] = pos2[n]
                Mmat[c, t, p, gid[n] - g_lo[c]] = 1.0

    GROWS = 640  # padded graph rows (>= 500 + 128 window slack)
    cnt = np.maximum(np.bincount(gid, minlength=GROWS).astype(np.float32), 1.0)

    return dict(
        Ncore=Ncore, TILES=TILES, NT=NT, K_t=K_t, S1=S1, off_t=off_t,
        T1ROWS=T1ROWS, dummy1=dummy1, dummy2=dummy2, GROWS=GROWS,
        idx1=idx1, idx2=idx2, own1=own1, own2=own2, Mmat=Mmat, scat=scat,
        cnt=cnt.reshape(GROWS, 1),
    )


# ----------------------------------------------------------------------------
# Bass program
# ----------------------------------------------------------------------------

def _build_program(N, F, Gn, C, pre):
    TILES = pre["TILES"]
    NT = pre["NT"]
    K_t = pre["K_t"]
    S1 = pre["S1"]
    off_t = pre["off_t"]
    T1ROWS = pre["T1ROWS"]
    GROWS = pre["GROWS"]

    HF = 2 * F                  # layer-1 output width (2 heads)
    ROW1 = HF + 8               # 208: feat(200) el(2) er(2) pad(4)
    GC1 = HF + 4                # gathered columns
    ROW2 = 128                  # feat2(100) el2(1) er2(1) pad
    NTS = NT + 1                # shard rows incl. trailing dummy row
    T2ROWS = NC * NTS

    nc = bacc.Bacc("TRN2", target_bir_lowering=False, debug=False,
                   enable_asserts=False, num_devices=NC)

    # ---- I/O tensors ----
    xT = [nc.dram_tensor(f"xT{b}", [F, T1ROWS], F32, kind="ExternalInput")
          for b in range(3)]
    W1 = nc.dram_tensor("W1", [F, HF], F32, kind="ExternalInput")
    al1 = nc.dram_tensor("al1", [2, F], F32, kind="ExternalInput")
    ar1 = nc.dram_tensor("ar1", [2, F], F32, kind="ExternalInput")
    b1 = nc.dram_tensor("b1", [HF], F32, kind="ExternalInput")
    W2 = nc.dram_tensor("W2", [HF, F], F32, kind="ExternalInput")
    al2 = nc.dram_tensor("al2", [1, F], F32, kind="ExternalInput")
    ar2 = nc.dram_tensor("ar2", [1, F], F32, kind="ExternalInput")
    b2 = nc.dram_tensor("b2", [F], F32, kind="ExternalInput")
    Wfc = nc.dram_tensor("Wfc", [3 * F, F], F32, kind="ExternalInput")
    bfc = nc.dram_tensor("bfc", [F], F32, kind="ExternalInput")
    Wcls = nc.dram_tensor("Wcls", [F, C], F32, kind="ExternalInput")
    bcls = nc.dram_tensor("bcls", [C], F32, kind="ExternalInput")
    idx1 = nc.dram_tensor("idx1", [P, S1], I32, kind="ExternalInput")
    idx2 = nc.dram_tensor("idx2", [P, S1], I32, kind="ExternalInput")
    own1 = nc.dram_tensor("own1", [P, TILES], I32, kind="ExternalInput")
    own2 = nc.dram_tensor("own2", [P, TILES], I32, kind="ExternalInput")
    Mm = nc.dram_tensor("Mm", [TILES, P, P], F32, kind="ExternalInput")
    scat = nc.dram_tensor("scat", [P, 1], I32, kind="ExternalInput")
    cnt = nc.dram_tensor("cnt", [GROWS, 1], F32, kind="ExternalInput")
    out = nc.dram_tensor("out", [Gn, C], F32, kind="ExternalOutput")

    def bcast(handle, n, parts=P):
        ap = handle.ap()
        return bass.AP(tensor=ap.tensor, offset=0, ap=[[0, parts], [1, n]])

    with tile.TileContext(nc) as tc, ExitStack() as ctx:
        sing = ctx.enter_context(tc.tile_pool(name="sing", bufs=1))
        xp = ctx.enter_context(tc.tile_pool(name="xp", bufs=2))
        ep = ctx.enter_context(tc.tile_pool(name="ep", bufs=2))
        sm = ctx.enter_context(tc.tile_pool(name="sm", bufs=3))
        hp = ctx.enter_context(tc.tile_pool(name="hp", bufs=2))
        mp = ctx.enter_context(tc.tile_pool(name="mp", bufs=2))
        pt1 = ctx.enter_context(tc.tile_pool(name="pt1", bufs=3, space="PSUM"))
        ptp = ctx.enter_context(tc.tile_pool(name="ptp", bufs=2, space="PSUM"))
        pt2 = ctx.enter_context(tc.tile_pool(name="pt2", bufs=2, space="PSUM"))
        pme = ctx.enter_context(tc.tile_pool(name="pme", bufs=1, space="PSUM"))
        dp = ctx.enter_context(tc.tile_pool(name="dp", bufs=2, space="DRAM"))
        dp1 = ctx.enter_context(tc.tile_pool(name="dp1", bufs=1, space="DRAM"))

        # ---------------- constants ----------------
        W1e = sing.tile([F, GC1], F32)           # [W1 | W1.al1 | W1.ar1]
        nc.sync.dma_start(out=W1e[:, 0:HF], in_=W1[:, :])
        tmp = sing.tile([F, HF], F32)
        attb = sing.tile([F, HF], F32)
        nc.sync.dma_start(out=attb[:], in_=bcast(al1, HF, F))
        nc.vector.tensor_tensor(out=tmp[:], in0=W1e[:, 0:HF], in1=attb[:],
                                op=ALU.mult)
        nc.vector.tensor_reduce(out=W1e[:, HF:HF + 2],
                                in_=tmp[:].rearrange("p (h f) -> p h f", h=2),
                                axis=mybir.AxisListType.X, op=ALU.add)
        nc.sync.dma_start(out=attb[:], in_=bcast(ar1, HF, F))
        nc.vector.tensor_tensor(out=tmp[:], in0=W1e[:, 0:HF], in1=attb[:],
                                op=ALU.mult)
        nc.vector.tensor_reduce(out=W1e[:, HF + 2:HF + 4],
                                in_=tmp[:].rearrange("p (h f) -> p h f", h=2),
                                axis=mybir.AxisListType.X, op=ALU.add)

        W2e = []                                  # two k-chunks of [W2|al2|ar2]
        tmp2 = sing.tile([F, F], F32)
        attb2 = sing.tile([F, F], F32)
        for j in range(2):
            w = sing.tile([F, F + 2], F32, tag=f"W2e{j}")
            nc.sync.dma_start(out=w[:, 0:F], in_=W2[j * F:(j + 1) * F, :])
            nc.sync.dma_start(out=attb2[:], in_=bcast(al2, F, F))
            nc.vector.tensor_tensor(out=tmp2[:], in0=w[:, 0:F], in1=attb2[:],
                                    op=ALU.mult)
            nc.vector.tensor_reduce(out=w[:, F:F + 1], in_=tmp2[:],
                                    axis=mybir.AxisListType.X, op=ALU.add)
            nc.sync.dma_start(out=attb2[:], in_=bcast(ar2, F, F))
            nc.vector.tensor_tensor(out=tmp2[:], in0=w[:, 0:F], in1=attb2[:],
                                    op=ALU.mult)
            nc.vector.tensor_reduce(out=w[:, F + 1:F + 2], in_=tmp2[:],
                                    axis=mybir.AxisListType.X, op=ALU.add)
            W2e.append(w)

        b1rep = sing.tile([P, HF], F32)
        nc.sync.dma_start(out=b1rep[:], in_=bcast(b1, HF))
        b2rep = sing.tile([P, F], F32)
        nc.sync.dma_start(out=b2rep[:], in_=bcast(b2, F))
        bfcrep = sing.tile([P, F], F32)
        nc.sync.dma_start(out=bfcrep[:], in_=bcast(bfc, F))
        bclsrep = sing.tile([P, C], F32)
        nc.sync.dma_start(out=bclsrep[:], in_=bcast(bcls, C))
        wfc_sb = sing.tile([F, 3 * F], F32)       # [k, j] chunks side by side
        for j in range(3):
            nc.sync.dma_start(out=wfc_sb[:, j * F:(j + 1) * F],
                              in_=Wfc[j * F:(j + 1) * F, :])
        wcls_sb = sing.tile([F, C], F32)
        nc.sync.dma_start(out=wcls_sb[:], in_=Wcls[:, :])
        ident = sing.tile([P, P], F32)
        make_identity(nc, ident[:])

        idx1sb = sing.tile([P, S1], I32)
        nc.sync.dma_start(out=idx1sb[:], in_=idx1[:, :])
        idx2sb = sing.tile([P, S1], I32)
        nc.sync.dma_start(out=idx2sb[:], in_=idx2[:, :])
        own1sb = sing.tile([P, TILES], I32)
        nc.sync.dma_start(out=own1sb[:], in_=own1[:, :])
        own2sb = sing.tile([P, TILES], I32)
        nc.sync.dma_start(out=own2sb[:], in_=own2[:, :])
        scatsb = sing.tile([P, 1], I32)
        nc.sync.dma_start(out=scatsb[:], in_=scat[:, :])

        dneg = sing.tile([1, 4], F32)
        nc.vector.memset(dneg[:], NEG_BIG)
        drow2 = sing.tile([1, ROW2], F32)
        nc.vector.memset(drow2[:], 0.0)
        nc.vector.memset(drow2[0:1, F:F + 1], NEG_BIG)

        partial = sing.tile([P, 3 * F], F32)

        # ---------------- per-branch pipeline ----------------
        XC = 3200  # node columns per xT chunk (25 tiles)
        T1TILES = T1ROWS // P

        for b in range(3):
            tb1 = dp.tile([T1ROWS, ROW1], F32, tag="table1")
            t2s = dp.tile([NTS, ROW2], F32, tag="t2shard")
            t2f = dp.tile([T2ROWS, ROW2], F32, tag="t2full", addr_space="Shared")

            # --- layer-1 node table: tb1 = xT.T @ W1e (full graph) ---
            t = 0
            for c0 in range(0, T1ROWS, XC):
                csz = min(XC, T1ROWS - c0)
                xc = xp.tile([F, XC], F32, tag="xc")
                nc.sync.dma_start(out=xc[:, 0:csz], in_=xT[b][:, c0:c0 + csz])
                for i in range(csz // P):
                    ps = pt1.tile([P, GC1], F32, tag="pt1")
                    nc.tensor.matmul(ps[:], lhsT=xc[:, i * P:(i + 1) * P],
                                     rhs=W1e[:], start=True, stop=True)
                    pc = xp.tile([P, GC1], F32, tag="t1c")
                    nc.scalar.activation(out=pc[:], in_=ps[:], func=AF.Copy,
                                         bias=0.0, scale=1.0)
                    nc.sync.dma_start(out=tb1[t * P:(t + 1) * P, 0:GC1],
                                      in_=pc[:])
                    t += 1
            nc.sync.dma_start(out=tb1[N:N + 1, HF:HF + 4], in_=dneg[:])

            # --- layer-1 edge phase + layer-2 table shard ---
            for t in range(TILES):
                K = int(K_t[t])
                o = int(off_t[t])
                G = ep.tile([P, K, GC1], F32, tag="G1")
                for k in range(K):
                    nc.gpsimd.indirect_dma_start(
                        out=G[:, k, :], out_offset=None, in_=tb1[:, :],
                        in_offset=bass.IndirectOffsetOnAxis(
                            ap=idx1sb[:, o + k:o + k + 1], axis=0))
                er = sm.tile([P, 2], F32, tag="er1")
                nc.gpsimd.indirect_dma_start(
                    out=er[:], out_offset=None, in_=tb1[:, :],
                    in_offset=bass.IndirectOffsetOnAxis(
                        ap=own1sb[:, t:t + 1], axis=0),
                    element_offset=HF + 2)
                acc = hp.tile([P, HF], F32, tag="acc1")
                nc.scalar.activation(out=acc[:], in_=b1rep[:], func=AF.Copy,
                                     bias=0.0, scale=1.0)
                for h in range(2):
                    z = sm.tile([P, K], F32, tag="z")
                    nc.scalar.activation(out=z[:], in_=G[:, :, HF + h],
                                         func=AF.Identity,
                                         bias=er[:, h:h + 1], scale=1.0)
                    e = sm.tile([P, K], F32, tag="e")
                    nc.vector.scalar_tensor_tensor(
                        out=e[:], in0=z[:], scalar=0.2, in1=z[:],
                        op0=ALU.mult, op1=ALU.max)
                    negm = sm.tile([P, 1], F32, tag="negm")
                    nc.vector.tensor_reduce(out=negm[:], in_=e[:],
                                            axis=mybir.AxisListType.X,
                                            op=ALU.max, negate=True)
                    a = sm.tile([P, K], F32, tag="a")
                    s = sm.tile([P, 1], F32, tag="s")
                    nc.scalar.activation(out=a[:], in_=e[:], func=AF.Exp,
                                         bias=negm[:, 0:1], scale=1.0,
                                         accum_out=s[:, 0:1])
                    rs = sm.tile([P, 1], F32, tag="rs")
                    nc.vector.reciprocal(out=rs[:], in_=s[:, 0:1])
                    al = sm.tile([P, K], F32, tag="al")
                    nc.scalar.activation(out=al[:], in_=a[:], func=AF.Identity,
                                         bias=0.0, scale=rs[:, 0:1])
                    for k in range(K):
                        nc.vector.scalar_tensor_tensor(
                            out=acc[:, h * F:(h + 1) * F],
                            in0=G[:, k, h * F:(h + 1) * F],
                            scalar=al[:, k:k + 1],
                            in1=acc[:, h * F:(h + 1) * F],
                            op0=ALU.mult, op1=ALU.add)
                # table-2 rows for this tile: (acc) @ [W2|al2|ar2]
                hTs = []
                for j in range(2):
                    tp = ptp.tile([P, P], F32, tag="ptp")
                    nc.tensor.transpose(tp[0:F, :], acc[:, j * F:(j + 1) * F],
                                        ident[:])
                    hT = hp.tile([F, P], F32, tag="hT")
                    nc.scalar.activation(out=hT[:], in_=tp[0:F, :],
                                         func=AF.Copy, bias=0.0, scale=1.0)
                    hTs.append(hT)
                ps2 = pt2.tile([P, F + 2], F32, tag="pt2")
                for j in range(2):
                    nc.tensor.matmul(ps2[:], lhsT=hTs[j][:], rhs=W2e[j][:],
                                     start=(j == 0), stop=(j == 1),
                                     skip_group_check=True)
                p2c = hp.tile([P, F + 2], F32, tag="t2c")
                nc.scalar.activation(out=p2c[:], in_=ps2[:], func=AF.Copy,
                                     bias=0.0, scale=1.0)
                nc.sync.dma_start(out=t2s[t * P:(t + 1) * P, 0:F + 2],
                                  in_=p2c[:])

            # --- halo exchange: AllGather the layer-2 table shards ---
            nc.sync.dma_start(out=t2s[NT:NT + 1, :], in_=drow2[:])
            nc.gpsimd.collective_compute(
                "AllGather", ALU.bypass,
                replica_groups=[list(range(NC))],
                ins=[t2s[:, :]], outs=[t2f[:, :]])

            # --- layer-2 edge phase + graph-mean matmul ---
            pm = pme.tile([P, F], F32, tag="pme")
            for t in range(TILES):
                K = int(K_t[t])
                o = int(off_t[t])
                G2 = ep.tile([P, K, ROW2], F32, tag="G2")
                for k in range(K):
                    nc.gpsimd.indirect_dma_start(
                        out=G2[:, k, :], out_offset=None, in_=t2f[:, :],
                        in_offset=bass.IndirectOffsetOnAxis(
                            ap=idx2sb[:, o + k:o + k + 1], axis=0))
                er2 = sm.tile([P, 1], F32, tag="er2")
                nc.gpsimd.indirect_dma_start(
                    out=er2[:], out_offset=None, in_=t2f[:, :],
                    in_offset=bass.IndirectOffsetOnAxis(
                        ap=own2sb[:, t:t + 1], axis=0),
                    element_offset=F + 1)
                acc2 = hp.tile([P, F], F32, tag="acc2")
                nc.scalar.activation(out=acc2[:], in_=b2rep[:], func=AF.Copy,
                                     bias=0.0, scale=1.0)
                z = sm.tile([P, K], F32, tag="z")
                nc.scalar.activation(out=z[:], in_=G2[:, :, F],
                                     func=AF.Identity,
                                     bias=er2[:, 0:1], scale=1.0)
                e = sm.tile([P, K], F32, tag="e")
                nc.vector.scalar_tensor_tensor(
                    out=e[:], in0=z[:], scalar=0.2, in1=z[:],
                    op0=ALU.mult, op1=ALU.max)
                negm = sm.tile([P, 1], F32, tag="negm")
                nc.vector.tensor_reduce(out=negm[:], in_=e[:],
                                        axis=mybir.AxisListType.X,
                                        op=ALU.max, negate=True)
                a = sm.tile([P, K], F32, tag="a")
                s = sm.tile([P, 1], F32, tag="s")
                nc.scalar.activation(out=a[:], in_=e[:], func=AF.Exp,
                                     bias=negm[:, 0:1], scale=1.0,
                                     accum_out=s[:, 0:1])
                rs = sm.tile([P, 1], F32, tag="rs")
                nc.vector.reciprocal(out=rs[:], in_=s[:, 0:1])
                al = sm.tile([P, K], F32, tag="al")
                nc.scalar.activation(out=al[:], in_=a[:], func=AF.Identity,
                                     bias=0.0, scale=rs[:, 0:1])
                for k in range(K):
                    nc.vector.scalar_tensor_tensor(
                        out=acc2[:], in0=G2[:, k, 0:F],
                        scalar=al[:, k:k + 1], in1=acc2[:],
                        op0=ALU.mult, op1=ALU.add)
                Mt = mp.tile([P, P], F32, tag="M")
                nc.sync.dma_start(out=Mt[:], in_=Mm[t, :, :])
                nc.tensor.matmul(pm[:], lhsT=Mt[:], rhs=acc2[:],
                                 start=(t == 0), stop=(t == TILES - 1),
                                 skip_group_check=True)
            nc.scalar.activation(out=partial[:, b * F:(b + 1) * F],
                                 in_=pm[:], func=AF.Copy, bias=0.0, scale=1.0)

        # ---------------- readout ----------------
        pf = dp1.tile([GROWS, 3 * F], F32, tag="pf")
        rsum = dp1.tile([GROWS, 3 * F], F32, tag="rsum", addr_space="Shared")
        zsb = sing.tile([P, 3 * F], F32)
        nc.vector.memset(zsb[:], 0.0)
        for j in range(GROWS // P):
            nc.sync.dma_start(out=pf[j * P:(j + 1) * P, :], in_=zsb[:])
        nc.gpsimd.indirect_dma_start(
            out=pf[:, :],
            out_offset=bass.IndirectOffsetOnAxis(ap=scatsb[:, 0:1], axis=0),
            in_=partial[:], in_offset=None)
        nc.gpsimd.collective_compute(
            "AllReduce", ALU.add, replica_groups=[list(range(NC))],
            ins=[pf[:, :]], outs=[rsum[:, :]])

        GT = (Gn + P - 1) // P
        for gt in range(GT):
            rt = hp.tile([P, 3 * F], F32, tag="rt")
            nc.sync.dma_start(out=rt[:], in_=rsum[gt * P:(gt + 1) * P, :])
            cntt = sm.tile([P, 1], F32, tag="cntt")
            nc.sync.dma_start(out=cntt[:], in_=cnt[gt * P:(gt + 1) * P, :])
            rc = sm.tile([P, 1], F32, tag="rc")
            nc.vector.reciprocal(out=rc[:], in_=cntt[:, 0:1])
            rbar = hp.tile([P, 3 * F], F32, tag="rbar")
            nc.scalar.activation(out=rbar[:], in_=rt[:], func=AF.Identity,
                                 bias=0.0, scale=rc[:, 0:1])
            rTs = []
            for j in range(3):
                tp = ptp.tile([P, P], F32, tag="ptp")
                nc.tensor.transpose(tp[0:F, :], rbar[:, j * F:(j + 1) * F],
                                    ident[:])
                rT = hp.tile([F, P], F32, tag=f"rT{j}")
                nc.scalar.activation(out=rT[:], in_=tp[0:F, :], func=AF.Copy,
                                     bias=0.0, scale=1.0)
                rTs.append(rT)
            psfc = pt2.tile([P, F], F32, tag="pt2")
            for j in range(3):
                nc.tensor.matmul(psfc[:], lhsT=rTs[j][:],
                                 rhs=wfc_sb[:, j * F:(j + 1) * F],
                                 start=(j == 0), stop=(j == 2),
                                 skip_group_check=True)
            tfc = hp.tile([P, F], F32, tag="tfc")
            nc.vector.tensor_tensor(out=tfc[:], in0=psfc[:], in1=bfcrep[:],
                                    op=ALU.add)
            trel = hp.tile([P, F], F32, tag="trel")
            nc.scalar.activation(out=trel[:], in_=tfc[:], func=AF.Relu,
                                 bias=0.0, scale=1.0)
            tpc = ptp.tile([P, P], F32, tag="ptp")
            nc.tensor.transpose(tpc[0:F, :], trel[:], ident[:])
            tT = hp.tile([F, P], F32, tag="hT")
            nc.scalar.activation(out=tT[:], in_=tpc[0:F, :], func=AF.Copy,
                                 bias=0.0, scale=1.0)
            pscls = pt2.tile([P, C], F32, tag="pt2")
            nc.tensor.matmul(pscls[:], lhsT=tT[:], rhs=wcls_sb[:],
                             start=True, stop=True)
            ocls = hp.tile([P, C], F32, tag="ocls")
            nc.vector.tensor_tensor(out=ocls[:], in0=pscls[:], in1=bclsrep[:],
                                    op=ALU.add)
            rows = min(P, Gn - gt * P)
            nc.sync.dma_start(out=out[gt * P:gt * P + rows, :],
                              in_=ocls[0:rows, :])

    nc.compile()
    return nc


# ----------------------------------------------------------------------------
# Entry point
# ----------------------------------------------------------------------------

def kernel(**inputs):
    global LAST_EXEC_NS
    x_pkt = np.ascontiguousarray(np.asarray(inputs["x_pkt"], np.float32))
    x_arv = np.ascontiguousarray(np.asarray(inputs["x_arv"], np.float32))
    x_stat = np.ascontiguousarray(np.asarray(inputs["x_stat"], np.float32))
    src = np.asarray(inputs["src"]).astype(np.int64)
    dst = np.asarray(inputs["dst"]).astype(np.int64)
    gid = np.asarray(inputs["graph_id"]).astype(np.int64)

    N, F = x_pkt.shape
    Gn = 500
    C = int(np.asarray(inputs["bcls"]).shape[0])

    pre = _preprocess(src, dst, gid, N)

    key = (N, F, Gn, C, pre["S1"], tuple(pre["K_t"].tolist()))
    if key not in _CACHE:
        _CACHE[key] = _build_program(N, F, Gn, C, pre)
    nc = _CACHE[key]

    T1ROWS = pre["T1ROWS"]

    def padT(x):
        xt = np.zeros((F, T1ROWS), np.float32)
        xt[:, :N] = x.T
        return xt

    common = {
        "xT0": padT(x_pkt), "xT1": padT(x_arv), "xT2": padT(x_stat),
        "W1": np.ascontiguousarray(np.asarray(inputs["W1"], np.float32)),
        "al1": np.ascontiguousarray(np.asarray(inputs["al1"], np.float32)),
        "ar1": np.ascontiguousarray(np.asarray(inputs["ar1"], np.float32)),
        "b1": np.ascontiguousarray(np.asarray(inputs["b1"], np.float32)),
        "W2": np.ascontiguousarray(np.asarray(inputs["W2"], np.float32)),
        "al2": np.ascontiguousarray(np.asarray(inputs["al2"], np.float32)),
        "ar2": np.ascontiguousarray(np.asarray(inputs["ar2"], np.float32)),
        "b2": np.ascontiguousarray(np.asarray(inputs["b2"], np.float32)),
        "Wfc": np.ascontiguousarray(np.asarray(inputs["Wfc"], np.float32)),
        "bfc": np.ascontiguousarray(np.asarray(inputs["bfc"], np.float32)),
        "Wcls": np.ascontiguousarray(np.asarray(inputs["Wcls"], np.float32)),
        "bcls": np.ascontiguousarray(np.asarray(inputs["bcls"], np.float32)),
        "cnt": pre["cnt"],
    }
    in_maps = []
    for c in range(NC):
        m = dict(common)
        m["idx1"] = pre["idx1"][c]
        m["idx2"] = pre["idx2"][c]
        m["own1"] = pre["own1"][c]
        m["own2"] = pre["own2"][c]
        m["Mm"] = pre["Mmat"][c]
        m["scat"] = pre["scat"][c]
        in_maps.append(m)

    trace = os.environ.get("GAT_TRACE", "0") == "1"
    if trace:
        _install_trace_shim()
    r = bass_utils.run_bass_kernel_spmd(nc, in_maps, core_ids=list(range(NC)),
                                        trace=trace)
    LAST_EXEC_NS = r.exec_time_ns
    return np.asarray(r.results[0]["out"], np.float32)


def _install_trace_shim():
    import sys, types, contextlib, ctypes
    if "antenv.axon_hooks" in sys.modules:
        return
    so_path = "/opt/axon/libaxon_pjrt.so"
    lib = ctypes.CDLL(so_path)
    if not hasattr(lib, "axon_start_nrt_profile"):
        return
    lib.axon_start_nrt_profile.argtypes = [ctypes.POINTER(ctypes.c_int64),
                                           ctypes.c_size_t]
    lib.axon_start_nrt_profile.restype = ctypes.c_int64
    lib.axon_stop_nrt_profile.argtypes = [ctypes.c_char_p]
    lib.axon_stop_nrt_profile.restype = ctypes.c_int64

    @contextlib.contextmanager
    def _hook(output_dir, device_ids):
        import jax
        jax.devices()
        if device_ids:
            ids = (ctypes.c_int64 * len(device_ids))(*device_ids)
            rc = lib.axon_start_nrt_profile(ids, len(device_ids))
        else:
            rc = lib.axon_start_nrt_profile(None, 0)
        if rc != 0:
            raise RuntimeError(f"axon_start_nrt_profile rc={rc}")
        try:
            yield
        finally:
            n = lib.axon_stop_nrt_profile(str(output_dir).encode())
            print(f"profile: {n} file(s) written to {output_dir}")

    mod = types.ModuleType("antenv.axon_hooks")
    mod.get_axon_ntff_profile_hook = lambda: _hook
    mod.set_axon_ntff_profile_hook = lambda h: None
    sys.modules["antenv.axon_hooks"] = mod
    bass_utils.upload_artifacts = lambda tmpdir: f"file://{tmpdir}"
